# revision 25
# baseline (speedup 1.0000x reference)
"""AnomalyTransformer forward pass on 8 TRN2 NeuronCores.

Sharding: pure data-parallel over batch B=8 -> one batch item per core.
All weights are replicated to every core; each core computes its batch
item's embedding, 3 encoder layers (attention + series/prior/sigma
outputs + FFN), final layernorm and projection, and writes its slice of
every output. No collectives are needed.

Per-core outputs: out [512,38], series/prior/sigma [3,8,512,512] f32.

Key implementation notes:
- Matmul operands are bf16 (weights host-cast; activations cast at the
  PSUM->SBUF copy); softmax/prior/LN math stays fp32.
- Attention head pairs share the PE array: K=64 score matmuls for the
  even/odd head go to row groups 0-63/64-127, the M=64 attn@V matmuls
  to column groups via tile_position, so pairs run concurrently.
- prior = c/sigma * exp(-d^2/(2 sigma^2)) with sigma <= 2.00002 by
  construction (3^(sigmoid+1e-5)-1), so prior underflows to exactly 0
  for |l-s| > 64; only a 256-wide band per 128-row chunk is computed,
  the rest of the staging tile is zeroed once (fixed slot->band layout).
- This walrus build accepts one semaphore wait per instruction;
  _split_waits moves overflow waits onto NoOps.
"""

import math
import os

import numpy as np

P = 128
WIN = 512
CIN = 38
COUT = 38
D = 512
H = 8
NL = 3
DFF = 512
EH = 64
NC4 = D // P  # 4 chunks of 128
B = 8
LN_EPS = 1e-3
INV_SQRT_2PI = 1.0 / math.sqrt(2.0 * math.pi)
LN3 = math.log(3.0)
SCALE = 1.0 / math.sqrt(EH)
BAND = 256  # prior band width per 128-row chunk

# band start column for each row chunk lc (rows lc*128..lc*128+127 need
# |l-s|<=64 -> s in [lc*128-64, lc*128+191], fits in [W0[lc], W0[lc]+256))
W0 = [0, 64, 192, 256]

last_exec_time_ns = None
last_results = None

_BUILT = {}


def _split_waits(nc, mybir):
    for func in nc.m.functions:
        for bb in func.blocks:
            new_instrs = []
            for ins in bb.instructions:
                w = (
                    list(ins.sync_info.on_wait)
                    if ins.sync_info and ins.sync_info.on_wait
                    else []
                )
                if len(w) > 1:
                    keep, overflow = w[:1], w[1:]
                    for ow in overflow:
                        nop = mybir.InstNoOp(
                            name=nc.get_next_instruction_name(),
                            engine=ins.engine,
                            ins=[],
                            outs=[],
                            sync_info=mybir.SyncInfo(on_wait=[ow], on_update=[]),
                        )
                        new_instrs.append(nop)
                    ins.sync_info.on_wait = keep
                new_instrs.append(ins)
            bb.instructions[:] = new_instrs


def _build_nc():
    import concourse.bass as bass
    import concourse.mybir as mybir
    from concourse import tile

    f32 = mybir.dt.float32
    bf16 = mybir.dt.bfloat16
    f32r = mybir.dt.float32r
    AF = mybir.ActivationFunctionType
    ALU = mybir.AluOpType

    nc = bass.Bass()
    gelu_af = (AF.Identity if os.environ.get("BASS_SIM_NO_GELU") else AF.Gelu)

    # Register const APs for the float biases used in activation().
    for val in (LN_EPS, LN3 * 1e-5):
        t = nc.alloc_sbuf_tensor(f"const-float32-{val}", [128, 1], f32)
        nc.gpsimd.memset(t.ap(), val)
        nc.const_aps.aps[(f32, val)] = t.ap()
    nc.all_engine_barrier()

    def dpi(name, shape, dt=f32):
        return nc.declare_dram_parameter(name, list(shape), dt, isOutput=False)

    # -------- inputs (per core; weights identical across cores) --------
    xT_d = dpi("xT", [CIN, WIN + 2])             # wrap-padded x, transposed
    ck_d = dpi("ck", [3, CIN, D])                # conv kernel f32
    Wq_d = dpi("Wq", [NL, D, D], bf16)
    Wk_d = dpi("Wk", [NL, D, D], bf16)
    Wv_d = dpi("Wv", [NL, D, D], bf16)
    Ws_d = dpi("Ws", [NL, D, H], bf16)
    Wo_d = dpi("Wo", [NL, D, D], bf16)
    W1_d = dpi("W1", [NL, D, DFF], bf16)
    W2_d = dpi("W2", [NL, DFF, D], bf16)
    Wp_d = dpi("Wp", [D, COUT], bf16)
    bq_d = dpi("bq", [NL, D])
    bk_d = dpi("bk", [NL, D])
    b1_d = dpi("b1", [NL, DFF])
    bs_bc_d = dpi("bs_bc", [NL, P, H])           # bs broadcast over partitions
    obias_d = dpi("obias", [NL, 2, P, D])        # (bo_eff, b2) broadcast
    lngb_d = dpi("lngb", [NL, 4, P, D])          # (g1,b1,g2,b2) broadcast
    fgb_d = dpi("fgb", [2, P, D])                # (gf, bf) broadcast
    bp_bc_d = dpi("bp_bc", [P, COUT])
    pos_d = dpi("pos", [P, NC4, D])              # positional emb, [p, lc, d]
    nd2_d = dpi("nd2", [P, NC4, BAND])           # -(l-s)^2/2 band, [p, lc, s-W0]
    ident_d = dpi("ident", [P, P])
    e2_d = dpi("E2", [2, P], bf16)               # head-pair expand matrix
    ones_d = dpi("ones1", [P, WIN])

    # -------- outputs --------
    out_d = nc.declare_dram_parameter("out", [WIN, COUT], f32, isOutput=True)
    ser_d = nc.declare_dram_parameter("series", [NL, H, WIN, WIN], f32, isOutput=True)
    pri_d = nc.declare_dram_parameter("prior", [NL, H, WIN, WIN], f32, isOutput=True)
    sig_d = nc.declare_dram_parameter("sigma", [NL, H, WIN, WIN], f32, isOutput=True)

    with tile.TileContext(nc) as tc:
        with (
            tc.tile_pool(name="consts", bufs=1) as cpool,
            tc.tile_pool(name="weights", bufs=2) as wpool,
            tc.tile_pool(name="woff", bufs=1) as wpool1,
            tc.tile_pool(name="bias", bufs=2) as bpool,
            tc.tile_pool(name="acts", bufs=1) as apool,
            tc.tile_pool(name="actd", bufs=2) as apool2,
            tc.tile_pool(name="expn", bufs=2) as epool,
            tc.tile_pool(name="outs", bufs=3) as opool,
            tc.tile_pool(name="outs1", bufs=2) as opool1,
            tc.tile_pool(name="psum", bufs=2, space="PSUM") as pp,
            tc.tile_pool(name="psum1", bufs=2, space="PSUM") as pp1,
        ):
            # ---------------- constants ----------------
            ident = cpool.tile([P, P], f32, tag="ident")
            nc.sync.dma_start(ident[:], ident_d[:])
            e2 = cpool.tile([2, P], bf16, tag="e2")
            nc.sync.dma_start(e2[:], e2_d[:])
            ones = cpool.tile([P, WIN], f32, tag="ones")
            nc.sync.dma_start(ones[:], ones_d[:])
            nd2 = cpool.tile([P, NC4, BAND], f32, tag="nd2")
            nc.sync.dma_start(nd2[:], nd2_d[:])
            pos = apool.tile([P, NC4, D], f32, tag="h1_nat")
            nc.sync.dma_start(pos[:], pos_d[:])
            bp_bc = cpool.tile([P, COUT], f32, tag="bp")
            nc.sync.dma_start(bp_bc[:], bp_bc_d[:])
            fgb = cpool.tile([P, 2, D], f32, tag="fgb")
            nc.sync.dma_start(fgb[:], fgb_d.rearrange("t p d -> p t d"))

            # prior staging tiles: fixed slot->(lh) mapping so the zeroed
            # outside-band region is written exactly once.
            pri_tiles = {}
            for lh in range(2):
                for sl in range(2):
                    pt = cpool.tile([P, 2, WIN], f32, tag=f"pri_out{lh}{sl}")
                    nc.gpsimd.memset(pt[:], 0.0)
                    pri_tiles[(lh, sl)] = pt

            # ---------------- embedding ----------------
            xT = apool.tile([CIN, WIN + 2], f32r, tag="xT")
            nc.sync.dma_start(xT[:], xT_d[:].bitcast(f32r))
            ck = apool.tile([CIN, 3, D], f32r, tag="ck")
            nc.sync.dma_start(ck[:], ck_d.rearrange("j c d -> c j d").bitcast(f32r))

            h_nat = apool.tile([P, NC4, D], f32, tag="h_nat")
            for lc in range(NC4):
                ps = pp.tile([P, D], f32, tag="proj")
                for j in range(3):
                    nc.tensor.matmul(
                        ps[:],
                        xT[:, j + lc * P : j + lc * P + P],
                        ck[:, j, :],
                        start=(j == 0),
                        stop=(j == 2),
                    )
                nc.vector.tensor_add(h_nat[:, lc, :], ps[:], pos[:, lc, :])

            def transpose_to(dst_bf16, src_f32):
                # src [P, NC4, D] natural -> dst [P, NC4, WIN] bf16 T-layout
                for dc in range(NC4):
                    pst = pp.tile([P, WIN], f32, tag="att", bufs=4)
                    for lc in range(NC4):
                        nc.tensor.transpose(
                            pst[:, lc * P : (lc + 1) * P],
                            src_f32[:, lc, dc * P : (dc + 1) * P],
                            ident[:],
                        )
                    nc.vector.tensor_copy(dst_bf16[:, dc, :], pst[:])

            def layer_norm(dst, src_psum, resid, obias_ap, g_ap, b_ap, lc):
                # t = src_psum + obias + resid ; dst[:, lc, :] = LN(t)*g+b
                t = apool2.tile([P, D], f32, tag="resid")
                nc.vector.scalar_tensor_tensor(
                    t[:], src_psum[:], 1.0, obias_ap, ALU.mult, ALU.add
                )
                nc.vector.tensor_add(t[:], t[:], resid[:, lc, :])
                stats = apool2.tile([P, 6], f32, tag="stats")
                nc.vector.bn_stats(stats[:], t[:])
                aggr = apool2.tile([P, 2], f32, tag="aggr")
                nc.vector.bn_aggr(aggr[:], stats[:])
                # rstd = exp(-0.5*ln(var+eps)) ; ln/exp share one ACT table set
                lnv = apool2.tile([P, 1], f32, tag="lnv")
                nc.scalar.activation(lnv[:], aggr[:, 1:2], AF.Ln, bias=LN_EPS)
                rstd = apool2.tile([P, 1], f32, tag="rstd")
                nc.scalar.activation(rstd[:], lnv[:], AF.Exp, scale=-0.5)
                negmu = apool2.tile([P, 1], f32, tag="negmu")
                nc.vector.tensor_scalar_mul(negmu[:], aggr[:, 0:1], -1.0)
                # dst = ((t - mu)*g)*rstd + b   (2 fused DVE passes)
                xh = apool2.tile([P, D], f32, tag="xh")
                nc.vector.scalar_tensor_tensor(
                    xh[:], t[:], negmu[:], g_ap, ALU.add, ALU.mult
                )
                nc.vector.scalar_tensor_tensor(
                    dst[:, lc, :], xh[:], rstd[:], b_ap, ALU.mult, ALU.add
                )

            for li in range(NL):
                # ---- layer weights ----
                wq = wpool.tile([P, NC4, D], bf16, tag="wq")
                nc.sync.dma_start(wq[:], Wq_d[li].rearrange("(c p) d -> p c d", p=P))
                wk = wpool.tile([P, NC4, D], bf16, tag="wk")
                nc.sync.dma_start(wk[:], Wk_d[li].rearrange("(c p) d -> p c d", p=P))
                wv = wpool.tile([P, NC4, D], bf16, tag="wv")
                nc.sync.dma_start(wv[:], Wv_d[li].rearrange("(c p) d -> p c d", p=P))
                ws = wpool.tile([P, NC4, H], bf16, tag="ws")
                nc.sync.dma_start(ws[:], Ws_d[li].rearrange("(c p) h -> p c h", p=P))
                wo = wpool1.tile([P, NC4, D], bf16, tag="wo")
                nc.scalar.dma_start(wo[:], Wo_d[li].rearrange("(c p) d -> p c d", p=P))
                w1 = wpool1.tile([P, NC4, DFF], bf16, tag="w1")
                nc.scalar.dma_start(w1[:], W1_d[li].rearrange("(c p) d -> p c d", p=P))
                w2 = wpool1.tile([P, NC4, D], bf16, tag="w2")
                nc.scalar.dma_start(w2[:], W2_d[li].rearrange("(c p) d -> p c d", p=P))
                bq = bpool.tile([P, NC4], f32, tag="bq")
                nc.sync.dma_start(bq[:], bq_d[li].rearrange("(c p) -> p c", p=P))
                bk = bpool.tile([P, NC4], f32, tag="bk")
                nc.sync.dma_start(bk[:], bk_d[li].rearrange("(c p) -> p c", p=P))
                b1 = bpool.tile([P, NC4], f32, tag="b1")
                nc.sync.dma_start(b1[:], b1_d[li].rearrange("(c p) -> p c", p=P))
                bs_bc = bpool.tile([P, H], f32, tag="bs")
                nc.sync.dma_start(bs_bc[:], bs_bc_d[li])
                obias = wpool1.tile([P, 2, D], f32, tag="obias")
                nc.scalar.dma_start(obias[:], obias_d[li].rearrange("t p d -> p t d"))
                lngb = wpool1.tile([P, 4, D], f32, tag="lngb")
                nc.scalar.dma_start(lngb[:], lngb_d[li].rearrange("t p d -> p t d"))

                # ---- hT = (h_nat)^T, bf16 ----
                hT = apool.tile([P, NC4, WIN], bf16, tag="hT")
                transpose_to(hT, h_nat)

                # ---- sigma projection + chain ----
                sg = apool2.tile([P, NC4, H], f32, tag="sg")
                c1 = apool2.tile([P, NC4, H], f32, tag="c1")
                isg2 = apool2.tile([P, NC4, H], f32, tag="isg2")
                for lc in range(NC4):
                    ps = pp.tile([P, H], f32, tag="proj")
                    for kc in range(NC4):
                        nc.tensor.matmul(
                            ps[:],
                            hT[:, kc, lc * P : (lc + 1) * P],
                            ws[:, kc, :],
                            start=(kc == 0),
                            stop=(kc == NC4 - 1),
                        )
                    t8 = apool2.tile([P, H], f32, tag="t8")
                    nc.vector.tensor_add(t8[:], ps[:], bs_bc[:])
                    # sigmoid(5x) = 1/(1+exp(-5x)); then sg = 3^(sig+1e-5)-1
                    e5 = apool2.tile([P, H], f32, tag="e5")
                    nc.scalar.activation(e5[:], t8[:], AF.Exp, scale=-5.0)
                    nc.vector.tensor_scalar_add(e5[:], e5[:], 1.0)
                    rcp = apool2.tile([P, H], f32, tag="rcp")
                    nc.vector.reciprocal(rcp[:], e5[:])
                    p3 = apool2.tile([P, H], f32, tag="p3")
                    nc.scalar.activation(
                        p3[:], rcp[:], AF.Exp, scale=LN3, bias=LN3 * 1e-5
                    )
                    nc.vector.tensor_scalar_add(sg[:, lc, :], p3[:], -1.0)
                    rsg = apool2.tile([P, H], f32, tag="rsg")
                    nc.vector.reciprocal(rsg[:], sg[:, lc, :])
                    nc.vector.tensor_scalar_mul(c1[:, lc, :], rsg[:], INV_SQRT_2PI)
                    nc.vector.tensor_mul(isg2[:, lc, :], rsg[:], rsg[:])

                def emit_prior_sigma(h, _li=li, _sg=None):
                    for lh in range(2):
                        st = opool1.tile(
                            [P, 2, WIN], f32, tag="sig_out",
                            name=f"sigst{_li}_{h}_{lh}",
                        )
                        pt = pri_tiles[(lh, h % 2)]
                        for l2 in range(2):
                            lc = lh * 2 + l2
                            nc.vector.tensor_scalar_mul(
                                st[:, l2, :], ones[:], sg[:, lc, h : h + 1]
                            )
                            pe = epool.tile(
                                [P, BAND], f32, tag="pri_exp",
                                name=f"pe{_li}_{h}_{lh}_{l2}",
                            )
                            nc.scalar.activation(
                                pe[:], nd2[:, lc, :], AF.Exp,
                                scale=isg2[:, lc, h : h + 1],
                            )
                            nc.vector.tensor_scalar_mul(
                                pt[:, l2, W0[lc] : W0[lc] + BAND], pe[:],
                                c1[:, lc, h : h + 1],
                            )
                        half_rows = slice(lh * 2 * P, (lh * 2 + 2) * P)
                        nc.scalar.dma_start(
                            sig_d[_li, h, half_rows, :].rearrange(
                                "(c p) s -> p c s", p=P
                            ),
                            st[:],
                        )
                        nc.sync.dma_start(
                            pri_d[_li, h, half_rows, :].rearrange(
                                "(c p) s -> p c s", p=P
                            ),
                            pt[:],
                        )

                # ---- q/k/v projections ----
                qT = apool.tile([P, NC4, WIN], bf16, tag="qT")
                kT = apool.tile([P, NC4, WIN], bf16, tag="kT")
                for dc in range(NC4):
                    psq = pp.tile([P, WIN], f32, tag="proj")
                    for kc in range(NC4):
                        nc.tensor.matmul(
                            psq[:],
                            wq[:, kc, dc * P : (dc + 1) * P],
                            hT[:, kc, :],
                            start=(kc == 0),
                            stop=(kc == NC4 - 1),
                        )
                    nc.vector.tensor_scalar_add(
                        qT[:, dc, :], psq[:], bq[:, dc : dc + 1]
                    )
                    psk = pp.tile([P, WIN], f32, tag="proj")
                    for kc in range(NC4):
                        nc.tensor.matmul(
                            psk[:],
                            wk[:, kc, dc * P : (dc + 1) * P],
                            hT[:, kc, :],
                            start=(kc == 0),
                            stop=(kc == NC4 - 1),
                        )
                    nc.vector.tensor_scalar_add(
                        kT[:, dc, :], psk[:], bk[:, dc : dc + 1]
                    )
                v = apool.tile([P, NC4, D], bf16, tag="v")
                for sc in range(NC4):
                    ps = pp.tile([P, D], f32, tag="proj")
                    for kc in range(NC4):
                        nc.tensor.matmul(
                            ps[:],
                            hT[:, kc, sc * P : (sc + 1) * P],
                            wv[:, kc, :],
                            start=(kc == 0),
                            stop=(kc == NC4 - 1),
                        )
                    nc.vector.tensor_copy(v[:, sc, :], ps[:])

                # ---- attention, one head pair (even/odd) per d-chunk ----
                rH = apool.tile([P, NC4, H], f32, tag="rH")
                attnT = apool.tile([P, NC4, WIN], bf16, tag="attnT")
                for dc in range(NC4):
                    heads = (2 * dc, 2 * dc + 1)
                    hps = (slice(0, 64), slice(64, 128))
                    pas = [
                        pp1.tile([P, WIN], f32, tag="attnps", name=f"pa{i}")
                        for i in range(2)
                    ]
                    # scores + series (halves adjacent -> PE row groups)
                    sums = [
                        apool2.tile([P, NC4], f32, tag="sums", name=f"sums{i}", bufs=4)
                        for i in range(2)
                    ]
                    for lh in range(2):
                        sts = [
                            opool.tile([P, 2, WIN], f32, tag="ser_out",
                                       name=f"st{i}")
                            for i in range(2)
                        ]
                        for l2 in range(2):
                            lc = lh * 2 + l2
                            pss = []
                            for half in range(2):
                                ps_s = pp.tile([P, WIN], f32, tag="att", bufs=4)
                                nc.tensor.matmul(
                                    ps_s[:],
                                    qT[hps[half], dc, lc * P : (lc + 1) * P],
                                    kT[hps[half], dc, :],
                                    start=True,
                                    stop=True,
                                )
                                pss.append(ps_s)
                            for half in range(2):
                                h = heads[half]
                                en = epool.tile([P, WIN], f32, tag="exp_nat", bufs=4)
                                nc.scalar.activation(
                                    en[:], pss[half][:], AF.Exp, scale=SCALE,
                                    accum_out=sums[half][:, lc : lc + 1],
                                )
                                nc.vector.reciprocal(
                                    rH[:, lc, h : h + 1],
                                    sums[half][:, lc : lc + 1],
                                )
                                nc.vector.tensor_scalar_mul(
                                    sts[half][:, l2, :], en[:],
                                    rH[:, lc, h : h + 1],
                                )
                        half_rows = slice(lh * 2 * P, (lh * 2 + 2) * P)
                        for half in range(2):
                            nc.sync.dma_start(
                                ser_d[li, heads[half], half_rows, :].rearrange(
                                    "(c p) s -> p c s", p=P
                                ),
                                sts[half][:],
                            )
                    # prior + sigma inline only for the first two pairs;
                    # heads 4-7 are emitted later to fill LN/FFN windows
                    if dc < 2:
                        emit_prior_sigma(heads[0])
                        emit_prior_sigma(heads[1])
                    # transposed scores -> expT (bf16), halves adjacent
                    eTs = [
                        apool2.tile([P, NC4, WIN], bf16, tag="eT", name=f"eT{i}", bufs=3)
                        for i in range(2)
                    ]
                    for sc in range(NC4):
                        pts = []
                        for half in range(2):
                            ps_t = pp.tile([P, WIN], f32, tag="att", bufs=4)
                            nc.tensor.matmul(
                                ps_t[:],
                                kT[hps[half], dc, sc * P : (sc + 1) * P],
                                qT[hps[half], dc, :],
                                start=True,
                                stop=True,
                            )
                            pts.append(ps_t)
                        for half in range(2):
                            nc.scalar.activation(
                                eTs[half][:, sc, :], pts[half][:], AF.Exp,
                                scale=SCALE,
                            )
                    # attn@V: column-group packed head pair (separate
                    # PSUM banks -- concurrent accumulation groups may not
                    # share a bank)
                    for sc in range(NC4):
                        for half in range(2):
                            h = heads[half]
                            nc.tensor.matmul(
                                pas[half][hps[half], :],
                                v[:, sc, 64 * h : 64 * h + 64],
                                eTs[half][:, sc, :],
                                start=(sc == 0),
                                stop=(sc == NC4 - 1),
                                tile_position=(0, 64 * half),
                            )
                    # normalize pair: r_bc = E2^T @ rT2 ; attnT = pa * r_bc
                    rt2 = apool2.tile([2, NC4, P], bf16, tag="rt2")
                    for lc in range(NC4):
                        ps_r = pp.tile([2, P], f32, tag="proj")
                        nc.tensor.transpose(
                            ps_r[:], rH[:, lc, 2 * dc : 2 * dc + 2], ident[:]
                        )
                        nc.vector.tensor_copy(rt2[:, lc, :], ps_r[:])
                    ps_b = pp.tile([P, WIN], f32, tag="proj")
                    nc.tensor.matmul(
                        ps_b[:], e2[:], rt2.rearrange("a b c -> a (b c)"),
                        start=True, stop=True,
                    )
                    rbc = apool2.tile([P, WIN], f32, tag="rbc_sb")
                    nc.vector.tensor_copy(rbc[:], ps_b[:])
                    for half in range(2):
                        nc.vector.tensor_mul(
                            attnT[hps[half], dc, :],
                            pas[half][hps[half], :],
                            rbc[hps[half], :],
                        )

                # ---- attention out-proj + LN1 ----
                h1_nat = apool.tile([P, NC4, D], f32, tag="h1_nat")
                for lc in range(NC4):
                    ps = pp.tile([P, D], f32, tag="proj")
                    for kc in range(NC4):
                        nc.tensor.matmul(
                            ps[:],
                            attnT[:, kc, lc * P : (lc + 1) * P],
                            wo[:, kc, :],
                            start=(kc == 0),
                            stop=(kc == NC4 - 1),
                        )
                    layer_norm(
                        h1_nat, ps, h_nat, obias[:, 0, :], lngb[:, 0, :],
                        lngb[:, 1, :], lc,
                    )

                emit_prior_sigma(4)
                emit_prior_sigma(5)

                # ---- FFN ----
                hT1 = apool.tile([P, NC4, WIN], bf16, tag="hT1")
                transpose_to(hT1, h1_nat)
                y1T = apool.tile([P, NC4, WIN], bf16, tag="y1T")
                for fc in range(NC4):
                    ps = pp.tile([P, WIN], f32, tag="proj")
                    for kc in range(NC4):
                        nc.tensor.matmul(
                            ps[:],
                            w1[:, kc, fc * P : (fc + 1) * P],
                            hT1[:, kc, :],
                            start=(kc == 0),
                            stop=(kc == NC4 - 1),
                        )
                    nc.scalar.activation(
                        y1T[:, fc, :], ps[:], gelu_af, bias=b1[:, fc : fc + 1]
                    )
                emit_prior_sigma(6)
                h_nat = apool.tile([P, NC4, D], f32, tag="h_nat")
                for lc in range(NC4):
                    ps = pp.tile([P, D], f32, tag="proj")
                    for fc in range(NC4):
                        nc.tensor.matmul(
                            ps[:],
                            y1T[:, fc, lc * P : (lc + 1) * P],
                            w2[:, fc, :],
                            start=(fc == 0),
                            stop=(fc == NC4 - 1),
                        )
                    layer_norm(
                        h_nat, ps, h1_nat, obias[:, 1, :], lngb[:, 2, :],
                        lngb[:, 3, :], lc,
                    )
                emit_prior_sigma(7)

            # ---------------- final LN + projection ----------------
            wp = wpool.tile([P, NC4, COUT], bf16, tag="wp")
            nc.sync.dma_start(wp[:], Wp_d.rearrange("(c p) d -> p c d", p=P))
            hf = apool.tile([P, NC4, D], f32, tag="h1_nat")
            for lc in range(NC4):
                stats = apool2.tile([P, 6], f32, tag="stats")
                nc.vector.bn_stats(stats[:], h_nat[:, lc, :])
                aggr = apool2.tile([P, 2], f32, tag="aggr")
                nc.vector.bn_aggr(aggr[:], stats[:])
                lnv = apool2.tile([P, 1], f32, tag="lnv")
                nc.scalar.activation(lnv[:], aggr[:, 1:2], AF.Ln, bias=LN_EPS)
                rstd = apool2.tile([P, 1], f32, tag="rstd")
                nc.scalar.activation(rstd[:], lnv[:], AF.Exp, scale=-0.5)
                negmu = apool2.tile([P, 1], f32, tag="negmu")
                nc.vector.tensor_scalar_mul(negmu[:], aggr[:, 0:1], -1.0)
                xh = apool2.tile([P, D], f32, tag="xh")
                nc.vector.scalar_tensor_tensor(
                    xh[:], h_nat[:, lc, :], negmu[:], fgb[:, 0, :],
                    ALU.add, ALU.mult,
                )
                nc.vector.scalar_tensor_tensor(
                    hf[:, lc, :], xh[:], rstd[:], fgb[:, 1, :],
                    ALU.mult, ALU.add,
                )
            hfT = apool.tile([P, NC4, WIN], bf16, tag="hT1")
            transpose_to(hfT, hf)
            for lc in range(NC4):
                ps = pp.tile([P, COUT], f32, tag="proj")
                for kc in range(NC4):
                    nc.tensor.matmul(
                        ps[:],
                        hfT[:, kc, lc * P : (lc + 1) * P],
                        wp[:, kc, :],
                        start=(kc == 0),
                        stop=(kc == NC4 - 1),
                    )
                ot = apool2.tile([P, COUT], f32, tag="ot")
                nc.vector.tensor_add(ot[:], ps[:], bp_bc[:])
                nc.sync.dma_start(out_d[lc * P : (lc + 1) * P, :], ot[:])

    _split_waits(nc, mybir)
    return nc


def _host_prep(inp):
    """Build the shared (weight/const) input map and per-core xT."""
    import ml_dtypes

    bf16 = ml_dtypes.bfloat16
    f32 = np.float32

    x = np.asarray(inp["x"], f32)
    shared = {}
    shared["ck"] = np.asarray(inp["conv_k"], f32)
    for nm in ["Wq", "Wk", "Wv", "Ws", "Wo", "W1", "W2", "Wp"]:
        shared[nm] = np.asarray(inp[nm], f32).astype(bf16)
    shared["bq"] = np.asarray(inp["bq"], f32)
    shared["bk"] = np.asarray(inp["bk"], f32)
    shared["b1"] = np.asarray(inp["b1"], f32)
    bs = np.asarray(inp["bs"], f32)
    shared["bs_bc"] = np.broadcast_to(bs[:, None, :], (NL, P, H)).copy()
    bo_eff = (np.einsum("ld,lde->le", np.asarray(inp["bv"], f32),
                        np.asarray(inp["Wo"], f32))
              + np.asarray(inp["bo"], f32))  # [NL, D]
    b2 = np.asarray(inp["b2"], f32)
    obias = np.stack([bo_eff, b2], axis=1)  # [NL, 2, D]
    shared["obias"] = np.broadcast_to(obias[:, :, None, :], (NL, 2, P, D)).copy()
    lngb = np.stack(
        [np.asarray(inp["ln1_g"], f32), np.asarray(inp["ln1_b"], f32),
         np.asarray(inp["ln2_g"], f32), np.asarray(inp["ln2_b"], f32)], axis=1
    )  # [NL, 4, D]
    shared["lngb"] = np.broadcast_to(lngb[:, :, None, :], (NL, 4, P, D)).copy()
    fgb = np.stack([np.asarray(inp["lnf_g"], f32), np.asarray(inp["lnf_b"], f32)])
    shared["fgb"] = np.broadcast_to(fgb[:, None, :], (2, P, D)).copy()
    shared["bp_bc"] = np.broadcast_to(
        np.asarray(inp["bp"], f32)[None, :], (P, COUT)
    ).copy()

    # positional embedding (matches reference._pos_embedding)
    lpos = np.arange(WIN, dtype=np.float64)[:, None]
    div = np.exp(np.arange(0, D, 2, dtype=np.float64) * (-math.log(10000.0) / D))
    pe = np.stack([np.sin(lpos * div), np.cos(lpos * div)], axis=-1).reshape(WIN, D)
    pe = pe.astype(f32)  # [l, d]
    shared["pos"] = pe.reshape(NC4, P, D).transpose(1, 0, 2).copy()

    # banded -(l-s)^2/2: for row l = lc*128+p, band cols s = W0[lc]+j
    idx = np.arange(WIN, dtype=np.float64)
    nd2_full = -0.5 * (idx[:, None] - idx[None, :]) ** 2  # [l, s]
    nd2 = np.zeros((P, NC4, BAND), f32)
    for lc in range(NC4):
        for p in range(P):
            nd2[p, lc, :] = nd2_full[lc * P + p, W0[lc] : W0[lc] + BAND]
    shared["nd2"] = nd2

    shared["ident"] = np.eye(P, dtype=f32)
    e2 = np.zeros((2, P), f32)
    e2[0, :64] = 1.0
    e2[1, 64:] = 1.0
    shared["E2"] = e2.astype(bf16)
    shared["ones1"] = np.ones((P, WIN), f32)

    in_maps = []
    for b in range(B):
        xp = np.concatenate([x[b, -1:], x[b], x[b, :1]], axis=0)  # [WIN+2, CIN]
        m = dict(shared)
        m["xT"] = np.ascontiguousarray(xp.T)  # [CIN, WIN+2]
        in_maps.append(m)
    return in_maps


def kernel(**inputs):
    global last_exec_time_ns, last_results
    trace = bool(os.environ.get("BASS_KERNEL_TRACE"))
    if trace:
        _install_trace_hook()

    if "nc" not in _BUILT:
        _BUILT["nc"] = _build_nc()
    nc = _BUILT["nc"]

    from concourse.bass_utils import run_bass_kernel_spmd

    in_maps = _host_prep(inputs)
    res = run_bass_kernel_spmd(nc, in_maps, list(range(B)), trace=trace)
    last_exec_time_ns = res.exec_time_ns
    last_results = res

    out = np.stack([res.results[b]["out"] for b in range(B)], axis=0)
    series = np.stack([res.results[b]["series"] for b in range(B)], axis=1)
    prior = np.stack([res.results[b]["prior"] for b in range(B)], axis=1)
    sigma = np.stack([res.results[b]["sigma"] for b in range(B)], axis=1)
    return out, series, prior, sigma


def _install_trace_hook():
    import sys
    import types

    import antenv

    if "antenv.axon_hooks" in sys.modules:
        return
    mod = types.ModuleType("antenv.axon_hooks")
    _hook = [None]
    mod.set_axon_ntff_profile_hook = lambda h: _hook.__setitem__(0, h)
    mod.get_axon_ntff_profile_hook = lambda: _hook[0]
    sys.modules["antenv.axon_hooks"] = mod
    antenv.axon_hooks = mod
    from trn_agent_boot.trn_boot import _ntff_profile_via_ctypes

    mod.set_axon_ntff_profile_hook(
        _ntff_profile_via_ctypes("/opt/axon/libaxon_pjrt.so")
    )
    from concourse import bass_utils

    bass_utils.upload_artifacts = lambda d: "local://" + d


# revision 26
# speedup vs baseline: 1.1078x; 1.1078x over previous
"""AnomalyTransformer forward pass on 8 TRN2 NeuronCores.

Sharding: pure data-parallel over batch B=8 -> one batch item per core.
All weights are replicated to every core; each core computes its batch
item's embedding, 3 encoder layers (attention + series/prior/sigma
outputs + FFN), final layernorm and projection, and writes its slice of
every output. No collectives are needed.

Per-core outputs: out [512,38], series/prior/sigma [3,8,512,512] f32.

Key implementation notes:
- Matmul operands are bf16 (weights host-cast; activations cast at the
  PSUM->SBUF copy); softmax/prior/LN math stays fp32.
- Attention head pairs share the PE array: K=64 score matmuls for the
  even/odd head go to row groups 0-63/64-127, the M=64 attn@V matmuls
  to column groups via tile_position, so pairs run concurrently.
- prior = c/sigma * exp(-d^2/(2 sigma^2)) with sigma <= 2.00002 by
  construction (3^(sigmoid+1e-5)-1), so prior underflows to exactly 0
  for |l-s| > 64; only a 256-wide band per 128-row chunk is computed,
  the rest of the staging tile is zeroed once (fixed slot->band layout).
- This walrus build accepts one semaphore wait per instruction;
  _split_waits moves overflow waits onto NoOps.
"""

import math
import os

import numpy as np

P = 128
WIN = 512
CIN = 38
COUT = 38
D = 512
H = 8
NL = 3
DFF = 512
EH = 64
NC4 = D // P  # 4 chunks of 128
B = 8
LN_EPS = 1e-3
INV_SQRT_2PI = 1.0 / math.sqrt(2.0 * math.pi)
LN3 = math.log(3.0)
SCALE = 1.0 / math.sqrt(EH)
BAND = 256  # prior band width per 128-row chunk

# band start column for each row chunk lc (rows lc*128..lc*128+127 need
# |l-s|<=64 -> s in [lc*128-64, lc*128+191], fits in [W0[lc], W0[lc]+256))
W0 = [0, 64, 192, 256]

last_exec_time_ns = None
last_results = None

_BUILT = {}


def _split_waits(nc, mybir):
    for func in nc.m.functions:
        for bb in func.blocks:
            new_instrs = []
            for ins in bb.instructions:
                w = (
                    list(ins.sync_info.on_wait)
                    if ins.sync_info and ins.sync_info.on_wait
                    else []
                )
                if len(w) > 1:
                    keep, overflow = w[:1], w[1:]
                    for ow in overflow:
                        nop = mybir.InstNoOp(
                            name=nc.get_next_instruction_name(),
                            engine=ins.engine,
                            ins=[],
                            outs=[],
                            sync_info=mybir.SyncInfo(on_wait=[ow], on_update=[]),
                        )
                        new_instrs.append(nop)
                    ins.sync_info.on_wait = keep
                new_instrs.append(ins)
            bb.instructions[:] = new_instrs


def _build_nc():
    import concourse.bass as bass
    import concourse.mybir as mybir
    from concourse import tile

    f32 = mybir.dt.float32
    bf16 = mybir.dt.bfloat16
    f32r = mybir.dt.float32r
    AF = mybir.ActivationFunctionType
    ALU = mybir.AluOpType

    nc = bass.Bass()
    gelu_af = (AF.Identity if os.environ.get("BASS_SIM_NO_GELU") else AF.Gelu)

    # Register const APs for the float biases used in activation().
    for val in (LN_EPS, LN3 * 1e-5):
        t = nc.alloc_sbuf_tensor(f"const-float32-{val}", [128, 1], f32)
        nc.gpsimd.memset(t.ap(), val)
        nc.const_aps.aps[(f32, val)] = t.ap()
    nc.all_engine_barrier()

    def dpi(name, shape, dt=f32):
        return nc.declare_dram_parameter(name, list(shape), dt, isOutput=False)

    # -------- inputs (per core; weights identical across cores) --------
    xT_d = dpi("xT", [CIN, WIN + 2])             # wrap-padded x, transposed
    ck_d = dpi("ck", [3, CIN, D])                # conv kernel f32
    Wq_d = dpi("Wq", [NL, D, D], bf16)
    Wk_d = dpi("Wk", [NL, D, D], bf16)
    Wv_d = dpi("Wv", [NL, D, D], bf16)
    Ws_d = dpi("Ws", [NL, D, H], bf16)
    Wo_d = dpi("Wo", [NL, D, D], bf16)
    W1_d = dpi("W1", [NL, D, DFF], bf16)
    W2_d = dpi("W2", [NL, DFF, D], bf16)
    Wp_d = dpi("Wp", [D, COUT], bf16)
    bq_d = dpi("bq", [NL, D])
    bk_d = dpi("bk", [NL, D])
    b1_d = dpi("b1", [NL, DFF])
    bs_bc_d = dpi("bs_bc", [NL, P, H])           # bs broadcast over partitions
    obias_d = dpi("obias", [NL, 2, P, D])        # (bo_eff, b2) broadcast
    lngb_d = dpi("lngb", [NL, 4, P, D])          # (g1,b1,g2,b2) broadcast
    fgb_d = dpi("fgb", [2, P, D])                # (gf, bf) broadcast
    bp_bc_d = dpi("bp_bc", [P, COUT])
    pos_d = dpi("pos", [P, NC4, D])              # positional emb, [p, lc, d]
    nd2_d = dpi("nd2", [P, NC4, BAND])           # -(l-s)^2/2 band, [p, lc, s-W0]
    ident_d = dpi("ident", [P, P])
    e2_d = dpi("E2", [2, P], bf16)               # head-pair expand matrix
    ones_d = dpi("ones1", [P, WIN])

    # -------- outputs --------
    out_d = nc.declare_dram_parameter("out", [WIN, COUT], f32, isOutput=True)
    ser_d = nc.declare_dram_parameter("series", [NL, H, WIN, WIN], f32, isOutput=True)
    pri_d = nc.declare_dram_parameter("prior", [NL, H, WIN, WIN], f32, isOutput=True)
    sig_d = nc.declare_dram_parameter("sigma", [NL, H, WIN, WIN], f32, isOutput=True)

    with tile.TileContext(nc) as tc:
        with (
            tc.tile_pool(name="consts", bufs=1) as cpool,
            tc.tile_pool(name="weights", bufs=2) as wpool,
            tc.tile_pool(name="woff", bufs=1) as wpool1,
            tc.tile_pool(name="bias", bufs=2) as bpool,
            tc.tile_pool(name="acts", bufs=1) as apool,
            tc.tile_pool(name="actd", bufs=2) as apool2,
            tc.tile_pool(name="expn", bufs=2) as epool,
            tc.tile_pool(name="outs", bufs=3) as opool,
            tc.tile_pool(name="outs1", bufs=2) as opool1,
            tc.tile_pool(name="psum", bufs=2, space="PSUM") as pp,
            tc.tile_pool(name="psum1", bufs=2, space="PSUM") as pp1,
        ):
            # ---------------- constants ----------------
            ident = cpool.tile([P, P], f32, tag="ident")
            nc.sync.dma_start(ident[:], ident_d[:])
            e2 = cpool.tile([2, P], bf16, tag="e2")
            nc.sync.dma_start(e2[:], e2_d[:])
            ones = cpool.tile([P, WIN], f32, tag="ones")
            nc.sync.dma_start(ones[:], ones_d[:])
            nd2 = cpool.tile([P, NC4, BAND], f32, tag="nd2")
            nc.sync.dma_start(nd2[:], nd2_d[:])
            pos = apool.tile([P, NC4, D], f32, tag="h1_nat")
            nc.sync.dma_start(pos[:], pos_d[:])
            bp_bc = cpool.tile([P, COUT], f32, tag="bp")
            nc.sync.dma_start(bp_bc[:], bp_bc_d[:])
            fgb = cpool.tile([P, 2, D], f32, tag="fgb")
            nc.sync.dma_start(fgb[:], fgb_d.rearrange("t p d -> p t d"))

            # prior staging tiles: fixed slot->(lh) mapping so the zeroed
            # outside-band region is written exactly once.
            pri_tiles = {}
            for lh in range(2):
                for sl in range(2):
                    pt = cpool.tile([P, 2, WIN], f32, tag=f"pri_out{lh}{sl}")
                    nc.gpsimd.memset(pt[:], 0.0)
                    pri_tiles[(lh, sl)] = pt

            # ---------------- embedding ----------------
            xT = apool.tile([CIN, WIN + 2], f32r, tag="xT")
            nc.sync.dma_start(xT[:], xT_d[:].bitcast(f32r))
            ck = apool.tile([CIN, 3, D], f32r, tag="ck")
            nc.sync.dma_start(ck[:], ck_d.rearrange("j c d -> c j d").bitcast(f32r))

            h_nat = apool.tile([P, NC4, D], f32, tag="h_nat")
            for lc in range(NC4):
                ps = pp.tile([P, D], f32, tag="proj")
                for j in range(3):
                    nc.tensor.matmul(
                        ps[:],
                        xT[:, j + lc * P : j + lc * P + P],
                        ck[:, j, :],
                        start=(j == 0),
                        stop=(j == 2),
                    )
                nc.vector.tensor_add(h_nat[:, lc, :], ps[:], pos[:, lc, :])

            def transpose_to(dst_bf16, src_f32):
                # src [P, NC4, D] natural -> dst [P, NC4, WIN] bf16 T-layout
                for dc in range(NC4):
                    pst = pp.tile([P, WIN], f32, tag="att", bufs=4)
                    for lc in range(NC4):
                        nc.tensor.transpose(
                            pst[:, lc * P : (lc + 1) * P],
                            src_f32[:, lc, dc * P : (dc + 1) * P],
                            ident[:],
                        )
                    nc.vector.tensor_copy(dst_bf16[:, dc, :], pst[:])

            def layer_norm(dst, src_psum, resid, obias_ap, g_ap, b_ap, lc):
                # t = src_psum + obias + resid ; dst[:, lc, :] = LN(t)*g+b
                t = apool2.tile([P, D], f32, tag="resid")
                nc.vector.scalar_tensor_tensor(
                    t[:], src_psum[:], 1.0, obias_ap, ALU.mult, ALU.add
                )
                nc.vector.tensor_add(t[:], t[:], resid[:, lc, :])
                stats = apool2.tile([P, 6], f32, tag="stats")
                nc.vector.bn_stats(stats[:], t[:])
                aggr = apool2.tile([P, 2], f32, tag="aggr")
                nc.vector.bn_aggr(aggr[:], stats[:])
                # rstd = exp(-0.5*ln(var+eps)) ; ln/exp share one ACT table set
                lnv = apool2.tile([P, 1], f32, tag="lnv")
                nc.scalar.activation(lnv[:], aggr[:, 1:2], AF.Ln, bias=LN_EPS)
                rstd = apool2.tile([P, 1], f32, tag="rstd")
                nc.scalar.activation(rstd[:], lnv[:], AF.Exp, scale=-0.5)
                negmu = apool2.tile([P, 1], f32, tag="negmu")
                nc.vector.tensor_scalar_mul(negmu[:], aggr[:, 0:1], -1.0)
                # dst = ((t - mu)*g)*rstd + b   (2 fused DVE passes)
                xh = apool2.tile([P, D], f32, tag="xh")
                nc.vector.scalar_tensor_tensor(
                    xh[:], t[:], negmu[:], g_ap, ALU.add, ALU.mult
                )
                nc.vector.scalar_tensor_tensor(
                    dst[:, lc, :], xh[:], rstd[:], b_ap, ALU.mult, ALU.add
                )

            for li in range(NL):
                # ---- layer weights ----
                wq = wpool.tile([P, NC4, D], bf16, tag="wq")
                nc.sync.dma_start(wq[:], Wq_d[li].rearrange("(c p) d -> p c d", p=P))
                wk = wpool.tile([P, NC4, D], bf16, tag="wk")
                nc.sync.dma_start(wk[:], Wk_d[li].rearrange("(c p) d -> p c d", p=P))
                wv = wpool.tile([P, NC4, D], bf16, tag="wv")
                nc.sync.dma_start(wv[:], Wv_d[li].rearrange("(c p) d -> p c d", p=P))
                ws = wpool.tile([P, NC4, H], bf16, tag="ws")
                nc.sync.dma_start(ws[:], Ws_d[li].rearrange("(c p) h -> p c h", p=P))
                wo = wpool1.tile([P, NC4, D], bf16, tag="wo")
                nc.scalar.dma_start(wo[:], Wo_d[li].rearrange("(c p) d -> p c d", p=P))
                w1 = wpool1.tile([P, NC4, DFF], bf16, tag="w1")
                nc.scalar.dma_start(w1[:], W1_d[li].rearrange("(c p) d -> p c d", p=P))
                w2 = wpool1.tile([P, NC4, D], bf16, tag="w2")
                nc.scalar.dma_start(w2[:], W2_d[li].rearrange("(c p) d -> p c d", p=P))
                bq = bpool.tile([P, NC4], f32, tag="bq")
                nc.sync.dma_start(bq[:], bq_d[li].rearrange("(c p) -> p c", p=P))
                bk = bpool.tile([P, NC4], f32, tag="bk")
                nc.sync.dma_start(bk[:], bk_d[li].rearrange("(c p) -> p c", p=P))
                b1 = bpool.tile([P, NC4], f32, tag="b1")
                nc.sync.dma_start(b1[:], b1_d[li].rearrange("(c p) -> p c", p=P))
                bs_bc = bpool.tile([P, H], f32, tag="bs")
                nc.sync.dma_start(bs_bc[:], bs_bc_d[li])
                obias = wpool1.tile([P, 2, D], f32, tag="obias")
                nc.scalar.dma_start(obias[:], obias_d[li].rearrange("t p d -> p t d"))
                lngb = wpool1.tile([P, 4, D], f32, tag="lngb")
                nc.scalar.dma_start(lngb[:], lngb_d[li].rearrange("t p d -> p t d"))

                # ---- hT = (h_nat)^T, bf16 ----
                hT = apool.tile([P, NC4, WIN], bf16, tag="hT")
                transpose_to(hT, h_nat)

                # ---- sigma projection + chain ----
                sg = apool2.tile([P, NC4, H], f32, tag="sg")
                c1 = apool2.tile([P, NC4, H], f32, tag="c1")
                isg2 = apool2.tile([P, NC4, H], f32, tag="isg2")
                for lc in range(NC4):
                    ps = pp.tile([P, H], f32, tag="proj")
                    for kc in range(NC4):
                        nc.tensor.matmul(
                            ps[:],
                            hT[:, kc, lc * P : (lc + 1) * P],
                            ws[:, kc, :],
                            start=(kc == 0),
                            stop=(kc == NC4 - 1),
                        )
                    t8 = apool2.tile([P, H], f32, tag="t8")
                    nc.vector.tensor_add(t8[:], ps[:], bs_bc[:])
                    # sigmoid(5x) = 1/(1+exp(-5x)); then sg = 3^(sig+1e-5)-1
                    e5 = apool2.tile([P, H], f32, tag="e5")
                    nc.scalar.activation(e5[:], t8[:], AF.Exp, scale=-5.0)
                    nc.vector.tensor_scalar_add(e5[:], e5[:], 1.0)
                    rcp = apool2.tile([P, H], f32, tag="rcp")
                    nc.vector.reciprocal(rcp[:], e5[:])
                    p3 = apool2.tile([P, H], f32, tag="p3")
                    nc.scalar.activation(
                        p3[:], rcp[:], AF.Exp, scale=LN3, bias=LN3 * 1e-5
                    )
                    nc.vector.tensor_scalar_add(sg[:, lc, :], p3[:], -1.0)
                    rsg = apool2.tile([P, H], f32, tag="rsg")
                    nc.vector.reciprocal(rsg[:], sg[:, lc, :])
                    nc.vector.tensor_scalar_mul(c1[:, lc, :], rsg[:], INV_SQRT_2PI)
                    nc.vector.tensor_mul(isg2[:, lc, :], rsg[:], rsg[:])

                def emit_prior_sigma(h, _li=li, _sg=None):
                    for lh in range(2):
                        st = opool1.tile(
                            [P, 2, WIN], f32, tag="sig_out",
                            name=f"sigst{_li}_{h}_{lh}",
                        )
                        pt = pri_tiles[(lh, h % 2)]
                        for l2 in range(2):
                            lc = lh * 2 + l2
                            nc.vector.tensor_scalar_mul(
                                st[:, l2, :], ones[:], sg[:, lc, h : h + 1]
                            )
                            pe = epool.tile(
                                [P, BAND], f32, tag="pri_exp",
                                name=f"pe{_li}_{h}_{lh}_{l2}",
                            )
                            nc.scalar.activation(
                                pe[:], nd2[:, lc, :], AF.Exp,
                                scale=isg2[:, lc, h : h + 1],
                            )
                            nc.vector.tensor_scalar_mul(
                                pt[:, l2, W0[lc] : W0[lc] + BAND], pe[:],
                                c1[:, lc, h : h + 1],
                            )
                        half_rows = slice(lh * 2 * P, (lh * 2 + 2) * P)
                        nc.scalar.dma_start(
                            sig_d[_li, h, half_rows, :].rearrange(
                                "(c p) s -> p c s", p=P
                            ),
                            st[:],
                        )
                        nc.sync.dma_start(
                            pri_d[_li, h, half_rows, :].rearrange(
                                "(c p) s -> p c s", p=P
                            ),
                            pt[:],
                        )

                # ---- q/k/v projections ----
                qT = apool.tile([P, NC4, WIN], bf16, tag="qT")
                kT = apool.tile([P, NC4, WIN], bf16, tag="kT")
                for dc in range(NC4):
                    psq = pp.tile([P, WIN], f32, tag="proj")
                    for kc in range(NC4):
                        nc.tensor.matmul(
                            psq[:],
                            wq[:, kc, dc * P : (dc + 1) * P],
                            hT[:, kc, :],
                            start=(kc == 0),
                            stop=(kc == NC4 - 1),
                        )
                    nc.vector.tensor_scalar_add(
                        qT[:, dc, :], psq[:], bq[:, dc : dc + 1]
                    )
                    psk = pp.tile([P, WIN], f32, tag="proj")
                    for kc in range(NC4):
                        nc.tensor.matmul(
                            psk[:],
                            wk[:, kc, dc * P : (dc + 1) * P],
                            hT[:, kc, :],
                            start=(kc == 0),
                            stop=(kc == NC4 - 1),
                        )
                    nc.vector.tensor_scalar_add(
                        kT[:, dc, :], psk[:], bk[:, dc : dc + 1]
                    )
                v = apool.tile([P, NC4, D], bf16, tag="v")
                for sc in range(NC4):
                    ps = pp.tile([P, D], f32, tag="proj")
                    for kc in range(NC4):
                        nc.tensor.matmul(
                            ps[:],
                            hT[:, kc, sc * P : (sc + 1) * P],
                            wv[:, kc, :],
                            start=(kc == 0),
                            stop=(kc == NC4 - 1),
                        )
                    nc.vector.tensor_copy(v[:, sc, :], ps[:])

                # ---- attention, one head pair (even/odd) per d-chunk ----
                rH = apool.tile([P, NC4, H], f32, tag="rH")
                attnT = apool.tile([P, NC4, WIN], bf16, tag="attnT")
                for dc in range(NC4):
                    heads = (2 * dc, 2 * dc + 1)
                    hps = (slice(0, 64), slice(64, 128))
                    pas = [
                        pp1.tile([P, WIN], f32, tag="attnps", name=f"pa{i}")
                        for i in range(2)
                    ]
                    # scores + series (halves adjacent -> PE row groups)
                    sums = [
                        apool2.tile([P, NC4], f32, tag="sums", name=f"sums{i}", bufs=4)
                        for i in range(2)
                    ]
                    for lh in range(2):
                        sts = [
                            opool.tile([P, 2, WIN], f32, tag="ser_out",
                                       name=f"st{i}")
                            for i in range(2)
                        ]
                        for l2 in range(2):
                            lc = lh * 2 + l2
                            pss = []
                            for half in range(2):
                                ps_s = pp.tile([P, WIN], f32, tag="att", bufs=4)
                                nc.tensor.matmul(
                                    ps_s[:],
                                    qT[hps[half], dc, lc * P : (lc + 1) * P],
                                    kT[hps[half], dc, :],
                                    start=True,
                                    stop=True,
                                )
                                pss.append(ps_s)
                            for half in range(2):
                                h = heads[half]
                                en = epool.tile([P, WIN], f32, tag="exp_nat", bufs=4)
                                nc.scalar.activation(
                                    en[:], pss[half][:], AF.Exp, scale=SCALE,
                                    accum_out=sums[half][:, lc : lc + 1],
                                )
                                nc.vector.reciprocal(
                                    rH[:, lc, h : h + 1],
                                    sums[half][:, lc : lc + 1],
                                )
                                nc.vector.tensor_scalar_mul(
                                    sts[half][:, l2, :], en[:],
                                    rH[:, lc, h : h + 1],
                                )
                        half_rows = slice(lh * 2 * P, (lh * 2 + 2) * P)
                        for half in range(2):
                            nc.sync.dma_start(
                                ser_d[li, heads[half], half_rows, :].rearrange(
                                    "(c p) s -> p c s", p=P
                                ),
                                sts[half][:],
                            )
                    emit_prior_sigma(heads[0])
                    emit_prior_sigma(heads[1])
                    # transposed scores -> expT (bf16), halves adjacent
                    eTs = [
                        apool2.tile([P, NC4, WIN], bf16, tag="eT", name=f"eT{i}", bufs=3)
                        for i in range(2)
                    ]
                    for sc in range(NC4):
                        pts = []
                        for half in range(2):
                            ps_t = pp.tile([P, WIN], f32, tag="att", bufs=4)
                            nc.tensor.matmul(
                                ps_t[:],
                                kT[hps[half], dc, sc * P : (sc + 1) * P],
                                qT[hps[half], dc, :],
                                start=True,
                                stop=True,
                            )
                            pts.append(ps_t)
                        for half in range(2):
                            nc.scalar.activation(
                                eTs[half][:, sc, :], pts[half][:], AF.Exp,
                                scale=SCALE,
                            )
                    # attn@V: column-group packed head pair (separate
                    # PSUM banks -- concurrent accumulation groups may not
                    # share a bank)
                    for sc in range(NC4):
                        for half in range(2):
                            h = heads[half]
                            nc.tensor.matmul(
                                pas[half][hps[half], :],
                                v[:, sc, 64 * h : 64 * h + 64],
                                eTs[half][:, sc, :],
                                start=(sc == 0),
                                stop=(sc == NC4 - 1),
                                tile_position=(0, 64 * half),
                            )
                    # normalize pair: r_bc = E2^T @ rT2 ; attnT = pa * r_bc
                    rt2 = apool2.tile([2, NC4, P], bf16, tag="rt2")
                    for lc in range(NC4):
                        ps_r = pp.tile([2, P], f32, tag="proj")
                        nc.tensor.transpose(
                            ps_r[:], rH[:, lc, 2 * dc : 2 * dc + 2], ident[:]
                        )
                        nc.vector.tensor_copy(rt2[:, lc, :], ps_r[:])
                    ps_b = pp.tile([P, WIN], f32, tag="proj")
                    nc.tensor.matmul(
                        ps_b[:], e2[:], rt2.rearrange("a b c -> a (b c)"),
                        start=True, stop=True,
                    )
                    rbc = apool2.tile([P, WIN], f32, tag="rbc_sb")
                    nc.vector.tensor_copy(rbc[:], ps_b[:])
                    for half in range(2):
                        nc.vector.tensor_mul(
                            attnT[hps[half], dc, :],
                            pas[half][hps[half], :],
                            rbc[hps[half], :],
                        )

                # ---- attention out-proj + LN1 ----
                h1_nat = apool.tile([P, NC4, D], f32, tag="h1_nat")
                for lc in range(NC4):
                    ps = pp.tile([P, D], f32, tag="proj")
                    for kc in range(NC4):
                        nc.tensor.matmul(
                            ps[:],
                            attnT[:, kc, lc * P : (lc + 1) * P],
                            wo[:, kc, :],
                            start=(kc == 0),
                            stop=(kc == NC4 - 1),
                        )
                    layer_norm(
                        h1_nat, ps, h_nat, obias[:, 0, :], lngb[:, 0, :],
                        lngb[:, 1, :], lc,
                    )

                # ---- FFN ----
                hT1 = apool.tile([P, NC4, WIN], bf16, tag="hT1")
                transpose_to(hT1, h1_nat)
                y1T = apool.tile([P, NC4, WIN], bf16, tag="y1T")
                for fc in range(NC4):
                    ps = pp.tile([P, WIN], f32, tag="proj")
                    for kc in range(NC4):
                        nc.tensor.matmul(
                            ps[:],
                            w1[:, kc, fc * P : (fc + 1) * P],
                            hT1[:, kc, :],
                            start=(kc == 0),
                            stop=(kc == NC4 - 1),
                        )
                    nc.scalar.activation(
                        y1T[:, fc, :], ps[:], gelu_af, bias=b1[:, fc : fc + 1]
                    )
                h_nat = apool.tile([P, NC4, D], f32, tag="h_nat")
                for lc in range(NC4):
                    ps = pp.tile([P, D], f32, tag="proj")
                    for fc in range(NC4):
                        nc.tensor.matmul(
                            ps[:],
                            y1T[:, fc, lc * P : (lc + 1) * P],
                            w2[:, fc, :],
                            start=(fc == 0),
                            stop=(fc == NC4 - 1),
                        )
                    layer_norm(
                        h_nat, ps, h1_nat, obias[:, 1, :], lngb[:, 2, :],
                        lngb[:, 3, :], lc,
                    )

            # ---------------- final LN + projection ----------------
            wp = wpool.tile([P, NC4, COUT], bf16, tag="wp")
            nc.sync.dma_start(wp[:], Wp_d.rearrange("(c p) d -> p c d", p=P))
            hf = apool.tile([P, NC4, D], f32, tag="h1_nat")
            for lc in range(NC4):
                stats = apool2.tile([P, 6], f32, tag="stats")
                nc.vector.bn_stats(stats[:], h_nat[:, lc, :])
                aggr = apool2.tile([P, 2], f32, tag="aggr")
                nc.vector.bn_aggr(aggr[:], stats[:])
                lnv = apool2.tile([P, 1], f32, tag="lnv")
                nc.scalar.activation(lnv[:], aggr[:, 1:2], AF.Ln, bias=LN_EPS)
                rstd = apool2.tile([P, 1], f32, tag="rstd")
                nc.scalar.activation(rstd[:], lnv[:], AF.Exp, scale=-0.5)
                negmu = apool2.tile([P, 1], f32, tag="negmu")
                nc.vector.tensor_scalar_mul(negmu[:], aggr[:, 0:1], -1.0)
                xh = apool2.tile([P, D], f32, tag="xh")
                nc.vector.scalar_tensor_tensor(
                    xh[:], h_nat[:, lc, :], negmu[:], fgb[:, 0, :],
                    ALU.add, ALU.mult,
                )
                nc.vector.scalar_tensor_tensor(
                    hf[:, lc, :], xh[:], rstd[:], fgb[:, 1, :],
                    ALU.mult, ALU.add,
                )
            hfT = apool.tile([P, NC4, WIN], bf16, tag="hT1")
            transpose_to(hfT, hf)
            for lc in range(NC4):
                ps = pp.tile([P, COUT], f32, tag="proj")
                for kc in range(NC4):
                    nc.tensor.matmul(
                        ps[:],
                        hfT[:, kc, lc * P : (lc + 1) * P],
                        wp[:, kc, :],
                        start=(kc == 0),
                        stop=(kc == NC4 - 1),
                    )
                ot = apool2.tile([P, COUT], f32, tag="ot")
                nc.vector.tensor_add(ot[:], ps[:], bp_bc[:])
                nc.sync.dma_start(out_d[lc * P : (lc + 1) * P, :], ot[:])

    _split_waits(nc, mybir)
    return nc


def _host_prep(inp):
    """Build the shared (weight/const) input map and per-core xT."""
    import ml_dtypes

    bf16 = ml_dtypes.bfloat16
    f32 = np.float32

    x = np.asarray(inp["x"], f32)
    shared = {}
    shared["ck"] = np.asarray(inp["conv_k"], f32)
    for nm in ["Wq", "Wk", "Wv", "Ws", "Wo", "W1", "W2", "Wp"]:
        shared[nm] = np.asarray(inp[nm], f32).astype(bf16)
    shared["bq"] = np.asarray(inp["bq"], f32)
    shared["bk"] = np.asarray(inp["bk"], f32)
    shared["b1"] = np.asarray(inp["b1"], f32)
    bs = np.asarray(inp["bs"], f32)
    shared["bs_bc"] = np.broadcast_to(bs[:, None, :], (NL, P, H)).copy()
    bo_eff = (np.einsum("ld,lde->le", np.asarray(inp["bv"], f32),
                        np.asarray(inp["Wo"], f32))
              + np.asarray(inp["bo"], f32))  # [NL, D]
    b2 = np.asarray(inp["b2"], f32)
    obias = np.stack([bo_eff, b2], axis=1)  # [NL, 2, D]
    shared["obias"] = np.broadcast_to(obias[:, :, None, :], (NL, 2, P, D)).copy()
    lngb = np.stack(
        [np.asarray(inp["ln1_g"], f32), np.asarray(inp["ln1_b"], f32),
         np.asarray(inp["ln2_g"], f32), np.asarray(inp["ln2_b"], f32)], axis=1
    )  # [NL, 4, D]
    shared["lngb"] = np.broadcast_to(lngb[:, :, None, :], (NL, 4, P, D)).copy()
    fgb = np.stack([np.asarray(inp["lnf_g"], f32), np.asarray(inp["lnf_b"], f32)])
    shared["fgb"] = np.broadcast_to(fgb[:, None, :], (2, P, D)).copy()
    shared["bp_bc"] = np.broadcast_to(
        np.asarray(inp["bp"], f32)[None, :], (P, COUT)
    ).copy()

    # positional embedding (matches reference._pos_embedding)
    lpos = np.arange(WIN, dtype=np.float64)[:, None]
    div = np.exp(np.arange(0, D, 2, dtype=np.float64) * (-math.log(10000.0) / D))
    pe = np.stack([np.sin(lpos * div), np.cos(lpos * div)], axis=-1).reshape(WIN, D)
    pe = pe.astype(f32)  # [l, d]
    shared["pos"] = pe.reshape(NC4, P, D).transpose(1, 0, 2).copy()

    # banded -(l-s)^2/2: for row l = lc*128+p, band cols s = W0[lc]+j
    idx = np.arange(WIN, dtype=np.float64)
    nd2_full = -0.5 * (idx[:, None] - idx[None, :]) ** 2  # [l, s]
    nd2 = np.zeros((P, NC4, BAND), f32)
    for lc in range(NC4):
        for p in range(P):
            nd2[p, lc, :] = nd2_full[lc * P + p, W0[lc] : W0[lc] + BAND]
    shared["nd2"] = nd2

    shared["ident"] = np.eye(P, dtype=f32)
    e2 = np.zeros((2, P), f32)
    e2[0, :64] = 1.0
    e2[1, 64:] = 1.0
    shared["E2"] = e2.astype(bf16)
    shared["ones1"] = np.ones((P, WIN), f32)

    in_maps = []
    for b in range(B):
        xp = np.concatenate([x[b, -1:], x[b], x[b, :1]], axis=0)  # [WIN+2, CIN]
        m = dict(shared)
        m["xT"] = np.ascontiguousarray(xp.T)  # [CIN, WIN+2]
        in_maps.append(m)
    return in_maps


def kernel(**inputs):
    global last_exec_time_ns, last_results
    trace = bool(os.environ.get("BASS_KERNEL_TRACE"))
    if trace:
        _install_trace_hook()

    if "nc" not in _BUILT:
        _BUILT["nc"] = _build_nc()
    nc = _BUILT["nc"]

    from concourse.bass_utils import run_bass_kernel_spmd

    in_maps = _host_prep(inputs)
    res = run_bass_kernel_spmd(nc, in_maps, list(range(B)), trace=trace)
    last_exec_time_ns = res.exec_time_ns
    last_results = res

    out = np.stack([res.results[b]["out"] for b in range(B)], axis=0)
    series = np.stack([res.results[b]["series"] for b in range(B)], axis=1)
    prior = np.stack([res.results[b]["prior"] for b in range(B)], axis=1)
    sigma = np.stack([res.results[b]["sigma"] for b in range(B)], axis=1)
    return out, series, prior, sigma


def _install_trace_hook():
    import sys
    import types

    import antenv

    if "antenv.axon_hooks" in sys.modules:
        return
    mod = types.ModuleType("antenv.axon_hooks")
    _hook = [None]
    mod.set_axon_ntff_profile_hook = lambda h: _hook.__setitem__(0, h)
    mod.get_axon_ntff_profile_hook = lambda: _hook[0]
    sys.modules["antenv.axon_hooks"] = mod
    antenv.axon_hooks = mod
    from trn_agent_boot.trn_boot import _ntff_profile_via_ctypes

    mod.set_axon_ntff_profile_hook(
        _ntff_profile_via_ctypes("/opt/axon/libaxon_pjrt.so")
    )
    from concourse import bass_utils

    bass_utils.upload_artifacts = lambda d: "local://" + d
